# revision 1
# baseline (speedup 1.0000x reference)
"""Trainium2 Bass kernel for MatrixMPowerSeriesLayer.

Computes res = sum_{i=0}^{7} C_i @ X^i for a batch of 64 complex 512x512
matrices (real/imag stacked), data-parallel over batch across 8 NeuronCores.

Algorithm (per batch element):
  Transposed Horner:  G_7 = C_7^T;  G_k = C_k^T + X^T @ G_{k+1}  (k = 6..0)
  => G_0 = res^T.  On the PE, out = lhsT.T @ rhs, so X^T @ G needs lhsT = X
  (untransposed!) and rhs = G: no transposes on device at all.  Host feeds
  coefficients transposed and transposes the result back.

  Complex multiply via Karatsuba (3 real matmuls instead of 4):
    T1 = Xr^T Gr, T2 = Xi^T Gi, T3 = (Xr+Xi)^T (Gr+Gi)
    real = Cr + T1 - T2, imag = Ci + T3 - T1 - T2
  The C-terms are folded into PSUM with identity-matmul "seeds":
    bank1 = T1
    bank2 = -Cr + T2        (seed rhs = -Cr^T, host-precomputed)
    bank3 = (Ci - Cr) + T3  (seed rhs = (Ci-Cr)^T, host-precomputed)
    Gr_new = bank1 - bank2                (1 DVE op)
    Gi_new = bank3 - bank1 - bank2        (2 DVE ops)
    Gs_new = Gr_new + Gi_new              (1 DVE op, rhs of next step's T3)

  All matmuls run as float32r (FP22): full bf16-rate on the PE at N=512
  with ~11 mantissa bits (measured end-to-end rel err ~5e-4 vs fp32).
"""

import numpy as np
from contextlib import ExitStack

import concourse.bass as bass
from concourse import bacc
import concourse.mybir as mybir
import concourse.tile as tile
from concourse.bass_utils import run_bass_kernel_spmd

B, N, DEG = 64, 512, 8
P = 128
KO = N // P          # 4 partition-chunks per 512 dim
NCORES = 8
BPC = B // NCORES    # 8 batch elements per core
F32 = mybir.dt.float32
F32R = mybir.dt.float32r

_NC_CACHE: dict = {}


def _build_nc(bpc: int = BPC, deg: int = DEG, coeff_bufs: int = 3) -> bass.Bass:
    """Build the per-core Bass program (SPMD; same program on all cores)."""
    nc = bacc.Bacc()

    # DRAM inputs (per core). Layout [*, KO, P, N]: row r of a 512x512 matrix
    # lives at [r // 128, r % 128, :], so each [P, N] chunk is contiguous.
    xr_d = nc.declare_dram_parameter("xr", [bpc, KO, P, N], F32R, isOutput=False)
    xi_d = nc.declare_dram_parameter("xi", [bpc, KO, P, N], F32R, isOutput=False)
    xs_d = nc.declare_dram_parameter("xs", [bpc, KO, P, N], F32R, isOutput=False)
    id_d = nc.declare_dram_parameter("ident", [P, P], F32R, isOutput=False)
    # Seeds for k = deg-2 .. 0 (index j corresponds to k = j)
    ncr_d = nc.declare_dram_parameter("ncr", [deg - 1, KO, P, N], F32R, isOutput=False)
    dct_d = nc.declare_dram_parameter("dct", [deg - 1, KO, P, N], F32R, isOutput=False)
    # Initial state G_{deg-1} = C_{deg-1}^T (shared by all batch elements)
    g0r_d = nc.declare_dram_parameter("g0r", [KO, P, N], F32R, isOutput=False)
    g0i_d = nc.declare_dram_parameter("g0i", [KO, P, N], F32R, isOutput=False)
    g0s_d = nc.declare_dram_parameter("g0s", [KO, P, N], F32R, isOutput=False)

    or_d = nc.declare_dram_parameter("o_r", [bpc, KO, P, N], F32R, isOutput=True)
    oi_d = nc.declare_dram_parameter("o_i", [bpc, KO, P, N], F32R, isOutput=True)

    with tile.TileContext(nc) as tc, ExitStack() as ctx:
        xp = ctx.enter_context(tc.tile_pool(name="xp", bufs=2))
        gp = ctx.enter_context(tc.tile_pool(name="gp", bufs=2))
        cp = ctx.enter_context(tc.tile_pool(name="cp", bufs=coeff_bufs))
        kp = ctx.enter_context(tc.tile_pool(name="kp", bufs=1))
        ps = ctx.enter_context(tc.tile_pool(name="ps", bufs=2, space="PSUM"))

        ident = kp.tile([P, P], F32R, name="ident")
        nc.sync.dma_start(out=ident[:], in_=id_d[:])
        ident_r = ident[:]

        for b in range(bpc):
            # Load this element's X tiles (lhsT operands; partition = k dim)
            xr_t = xp.tile([P, KO, N], F32R, tag="xr", name=f"xr{b}")
            xi_t = xp.tile([P, KO, N], F32R, tag="xi", name=f"xi{b}")
            xs_t = xp.tile([P, KO, N], F32R, tag="xs", name=f"xs{b}")
            for ko in range(KO):
                nc.sync.dma_start(out=xr_t[:, ko, :], in_=xr_d[b, ko])
                nc.sync.dma_start(out=xi_t[:, ko, :], in_=xi_d[b, ko])
                nc.sync.dma_start(out=xs_t[:, ko, :], in_=xs_d[b, ko])

            # Init Horner state G = C_{deg-1}^T
            gr = gp.tile([P, KO, N], F32R, tag="gr", name=f"gr{b}_init")
            gi = gp.tile([P, KO, N], F32R, tag="gi", name=f"gi{b}_init")
            gs = gp.tile([P, KO, N], F32R, tag="gs", name=f"gs{b}_init")
            for ko in range(KO):
                nc.sync.dma_start(out=gr[:, ko, :], in_=g0r_d[ko])
                nc.sync.dma_start(out=gi[:, ko, :], in_=g0i_d[ko])
                nc.sync.dma_start(out=gs[:, ko, :], in_=g0s_d[ko])

            for k in range(deg - 2, -1, -1):
                last = k == 0
                ncr_t = cp.tile([P, KO, N], F32R, tag="ncr", name=f"ncr{b}_{k}")
                dct_t = cp.tile([P, KO, N], F32R, tag="dct", name=f"dct{b}_{k}")
                for ko in range(KO):
                    nc.sync.dma_start(out=ncr_t[:, ko, :], in_=ncr_d[k, ko])
                    nc.sync.dma_start(out=dct_t[:, ko, :], in_=dct_d[k, ko])

                gr_n = gp.tile([P, KO, N], F32R, tag="gr", name=f"gr{b}_{k}")
                gi_n = gp.tile([P, KO, N], F32R, tag="gi", name=f"gi{b}_{k}")
                gs_n = (
                    None
                    if last
                    else gp.tile([P, KO, N], F32R, tag="gs", name=f"gs{b}_{k}")
                )

                for m in range(KO):
                    msl = slice(m * P, (m + 1) * P)
                    t1 = ps.tile([P, N], F32, tag="t1", name=f"t1_{b}_{k}_{m}")
                    t2 = ps.tile([P, N], F32, tag="t2", name=f"t2_{b}_{k}_{m}")
                    t3 = ps.tile([P, N], F32, tag="t3", name=f"t3_{b}_{k}_{m}")

                    # bank1 = T1 = (Xr^T Gr)[m]
                    for ko in range(KO):
                        nc.tensor.matmul(
                            t1[:],
                            lhsT=xr_t[:, ko, msl],
                            rhs=gr[:, ko, :],
                            start=(ko == 0),
                            stop=(ko == KO - 1),
                        )
                    # bank2 = -Cr^T[m] + T2[m]
                    nc.tensor.matmul(
                        t2[:],
                        lhsT=ident_r,
                        rhs=ncr_t[:, m, :],
                        start=True,
                        stop=False,
                    )
                    for ko in range(KO):
                        nc.tensor.matmul(
                            t2[:],
                            lhsT=xi_t[:, ko, msl],
                            rhs=gi[:, ko, :],
                            start=False,
                            stop=(ko == KO - 1),
                        )
                    # bank3 = (Ci-Cr)^T[m] + T3[m]
                    nc.tensor.matmul(
                        t3[:],
                        lhsT=ident_r,
                        rhs=dct_t[:, m, :],
                        start=True,
                        stop=False,
                    )
                    for ko in range(KO):
                        nc.tensor.matmul(
                            t3[:],
                            lhsT=xs_t[:, ko, msl],
                            rhs=gs[:, ko, :],
                            start=False,
                            stop=(ko == KO - 1),
                        )

                    # DVE may read only ONE PSUM operand per op: stage T1 to
                    # SBUF on ScalarE, then chain single-PSUM DVE ops.
                    u = kp.tile([P, N], F32, tag="u", bufs=3, name=f"u_{b}_{k}_{m}")
                    nc.scalar.copy(u[:], t1[:])
                    # Gr_new[m] = T1 - bank2 = Cr + T1 - T2
                    nc.vector.tensor_sub(gr_n[:, m, :], u[:], t2[:])
                    # Gi_new[m] = bank3 - T1 - bank2 = Ci + T3 - T1 - T2
                    nc.vector.tensor_sub(gi_n[:, m, :], t3[:], u[:])
                    nc.vector.tensor_sub(gi_n[:, m, :], gi_n[:, m, :], t2[:])
                    if last:
                        nc.sync.dma_start(out=or_d[b, m], in_=gr_n[:, m, :])
                        nc.sync.dma_start(out=oi_d[b, m], in_=gi_n[:, m, :])
                    else:
                        nc.vector.tensor_add(gs_n[:, m, :], gr_n[:, m, :], gi_n[:, m, :])

                gr, gi, gs = gr_n, gi_n, gs_n

    nc.finalize()
    return nc


def _get_nc() -> bass.Bass:
    if "nc" not in _NC_CACHE:
        _NC_CACHE["nc"] = _build_nc()
    return _NC_CACHE["nc"]


def _prep_inputs(x: np.ndarray, coeffs: np.ndarray):
    """Host-side prep: tile/transpose into the DRAM layouts the kernel wants."""
    x = np.ascontiguousarray(x, dtype=np.float32)
    coeffs = np.ascontiguousarray(coeffs, dtype=np.float32)

    xr = x[:, 0].reshape(B, KO, P, N)
    xi = x[:, 1].reshape(B, KO, P, N)
    xs = (x[:, 0] + x[:, 1]).reshape(B, KO, P, N)

    crT = np.ascontiguousarray(coeffs[:, 0].transpose(0, 2, 1))  # [DEG, N, N]
    ciT = np.ascontiguousarray(coeffs[:, 1].transpose(0, 2, 1))
    ncr = np.ascontiguousarray(-crT[: DEG - 1]).reshape(DEG - 1, KO, P, N)
    dct = np.ascontiguousarray(ciT[: DEG - 1] - crT[: DEG - 1]).reshape(
        DEG - 1, KO, P, N
    )
    g0r = np.ascontiguousarray(crT[DEG - 1]).reshape(KO, P, N)
    g0i = np.ascontiguousarray(ciT[DEG - 1]).reshape(KO, P, N)
    g0s = np.ascontiguousarray(crT[DEG - 1] + ciT[DEG - 1]).reshape(KO, P, N)

    ident = np.eye(P, dtype=np.float32)

    in_maps = []
    for c in range(NCORES):
        sl = slice(c * BPC, (c + 1) * BPC)
        in_maps.append(
            {
                "ident": ident,
                "xr": np.ascontiguousarray(xr[sl]),
                "xi": np.ascontiguousarray(xi[sl]),
                "xs": np.ascontiguousarray(xs[sl]),
                "ncr": ncr,
                "dct": dct,
                "g0r": g0r,
                "g0i": g0i,
                "g0s": g0s,
            }
        )
    return in_maps


def _assemble_output(results) -> np.ndarray:
    out = np.empty((B, 2, N, N), dtype=np.float32)
    for c in range(NCORES):
        o_r = results[c]["o_r"].reshape(BPC, N, N)
        o_i = results[c]["o_i"].reshape(BPC, N, N)
        for b in range(BPC):
            out[c * BPC + b, 0] = o_r[b].T
            out[c * BPC + b, 1] = o_i[b].T
    return out


def run_sharded(x: np.ndarray, coeffs: np.ndarray, **run_kwargs):
    """Run the SPMD kernel on 8 cores; returns (output, BassKernelResults)."""
    nc = _get_nc()
    in_maps = _prep_inputs(x, coeffs)
    res = run_bass_kernel_spmd(nc, in_maps, list(range(NCORES)), **run_kwargs)
    return _assemble_output(res.results), res


def kernel(x: np.ndarray, coeffs: np.ndarray) -> np.ndarray:
    out, _ = run_sharded(x, coeffs)
    return out



# revision 2
# speedup vs baseline: 1.2792x; 1.2792x over previous
"""Trainium2 Bass kernel for MatrixMPowerSeriesLayer.

Computes res = sum_{i=0}^{7} C_i @ X^i for a batch of 64 complex 512x512
matrices (real/imag stacked), data-parallel over batch across 8 NeuronCores.

Algorithm (per batch element):
  Transposed Horner:  G_7 = C_7^T;  G_k = C_k^T + X^T @ G_{k+1}  (k = 6..0)
  => G_0 = res^T.  On the PE, out = lhsT.T @ rhs, so X^T @ G needs lhsT = X
  (untransposed!) and rhs = G: no transposes on device at all.  Host feeds
  coefficients transposed and transposes the result back.

  Complex multiply via Karatsuba (3 real matmuls instead of 4):
    T1 = Xr^T Gr, T2 = Xi^T Gi, T3 = (Xr+Xi)^T (Gr+Gi)
    Gr_new = Cr^T + T1 - T2
    Gi_new = Ci^T + T3 - T1 - T2
    Gs_new = Gr_new + Gi_new            (rhs of next step's T3)

  v2 design (vs the identity-seed baseline):
  - fp16 operands everywhere: PE runs at full 2-byte rate AND FastWeightLoad
    kicks in (fp32 weights disabled it), so the per-matmul LDWEIGHTS hides
    under the previous matmul's 512-cycle stream.
  - No identity-seed matmuls: the C-terms ride on the ScalarE/VectorE combine
    instead, cutting PE work from 14 to 12 matmuls per output chunk.
  - ScalarE stages each PSUM bank to SBUF as fp16 (it sits closest to PSUM
    and is otherwise idle); VectorE then runs the whole Karatsuba combine as
    pure-fp16 SBUF ops, which hit the DVE 2x_1P perf mode (all-2-byte dtypes).
  - Two batch elements interleaved at the Horner-step level so the PE never
    waits on the combine tail at a step boundary.
"""

import numpy as np
from contextlib import ExitStack

import concourse.bass as bass
from concourse import bacc
import concourse.mybir as mybir
import concourse.tile as tile
from concourse.bass_utils import run_bass_kernel_spmd

B, N, DEG = 64, 512, 8
P = 128
KO = N // P          # 4 partition-chunks per 512 dim
NCORES = 8
BPC = B // NCORES    # 8 batch elements per core
F32 = mybir.dt.float32
F16 = mybir.dt.float16

_NC_CACHE: dict = {}


def _build_nc(bpc: int = BPC, deg: int = DEG) -> bass.Bass:
    """Build the per-core Bass program (SPMD; same program on all cores)."""
    nc = bacc.Bacc()

    # DRAM inputs (per core), fp16.  Layout [P, KO, N]: matrix row r lives at
    # [r % 128, r // 128, :], so each partition line is (KO, N) = 4 KB contig
    # and a whole matrix moves in ONE dma.
    xr_d = nc.declare_dram_parameter("xr", [bpc, P, KO, N], F16, isOutput=False)
    xi_d = nc.declare_dram_parameter("xi", [bpc, P, KO, N], F16, isOutput=False)
    xs_d = nc.declare_dram_parameter("xs", [bpc, P, KO, N], F16, isOutput=False)
    # Coefficients, j = deg-2-k (j=0 is the first Horner step k=deg-2)
    ctr_d = nc.declare_dram_parameter("ctr", [deg - 1, P, KO, N], F16, isOutput=False)
    cti_d = nc.declare_dram_parameter("cti", [deg - 1, P, KO, N], F16, isOutput=False)
    # Initial state G_{deg-1} = C_{deg-1}^T: [r, i, r+i]
    g0_d = nc.declare_dram_parameter("g0", [3, P, KO, N], F16, isOutput=False)

    or_d = nc.declare_dram_parameter("o_r", [bpc, KO, P, N], F16, isOutput=True)
    oi_d = nc.declare_dram_parameter("o_i", [bpc, KO, P, N], F16, isOutput=True)

    with tile.TileContext(nc) as tc, ExitStack() as ctx:
        xp = ctx.enter_context(tc.tile_pool(name="xp", bufs=2))
        gp = ctx.enter_context(tc.tile_pool(name="gp", bufs=2))
        kp = ctx.enter_context(tc.tile_pool(name="kp", bufs=1))
        sp = ctx.enter_context(tc.tile_pool(name="sp", bufs=3))
        vp = ctx.enter_context(tc.tile_pool(name="vp", bufs=2))
        ps = ctx.enter_context(tc.tile_pool(name="ps", bufs=2, space="PSUM"))

        ct_r = kp.tile([P, deg - 1, KO, N], F16, name="ct_r")
        ct_i = kp.tile([P, deg - 1, KO, N], F16, name="ct_i")
        g0_t = kp.tile([P, 3, KO, N], F16, name="g0_t")

        # Prologue DMAs in first-use order: G0 chunks + first coeff step.
        nc.sync.dma_start(out=g0_t[:, 0], in_=g0_d[0])
        nc.sync.dma_start(out=g0_t[:, 1], in_=g0_d[1])
        nc.sync.dma_start(out=g0_t[:, 2], in_=g0_d[2])
        nc.sync.dma_start(out=ct_r[:, 0], in_=ctr_d[0])
        nc.sync.dma_start(out=ct_i[:, 0], in_=cti_d[0])

        for pair in range(bpc // 2):
            b0, b1 = 2 * pair, 2 * pair + 1
            xts = {}
            for b in (b0, b1):
                par = b % 2
                xr_t = xp.tile([P, KO, N], F16, tag=f"xr{par}", name=f"xr{b}")
                xi_t = xp.tile([P, KO, N], F16, tag=f"xi{par}", name=f"xi{b}")
                xs_t = xp.tile([P, KO, N], F16, tag=f"xs{par}", name=f"xs{b}")
                nc.sync.dma_start(out=xr_t[:], in_=xr_d[b])
                nc.sync.dma_start(out=xi_t[:], in_=xi_d[b])
                nc.sync.dma_start(out=xs_t[:], in_=xs_d[b])
                xts[b] = (xr_t, xi_t, xs_t)
            if pair == 0:
                # Remaining coefficient steps stream in behind the X tiles;
                # step j is needed ~21*j us into the run, DMAs keep well ahead.
                for j in range(1, deg - 1):
                    nc.sync.dma_start(out=ct_r[:, j], in_=ctr_d[j])
                    nc.sync.dma_start(out=ct_i[:, j], in_=cti_d[j])

            # Current G state per element; k = deg-1 reads from shared g0_t.
            gcur = {b0: None, b1: None}

            for k in range(deg - 2, -1, -1):
                j = deg - 2 - k
                last = k == 0
                for b in (b0, b1):
                    par = b % 2
                    xr_t, xi_t, xs_t = xts[b]
                    if gcur[b] is None:
                        rhs_r = lambda ko: g0_t[:, 0, ko, :]
                        rhs_i = lambda ko: g0_t[:, 1, ko, :]
                        rhs_s = lambda ko: g0_t[:, 2, ko, :]
                    else:
                        _gr, _gi, _gs = gcur[b]
                        rhs_r = lambda ko, t=_gr: t[:, ko, :]
                        rhs_i = lambda ko, t=_gi: t[:, ko, :]
                        rhs_s = lambda ko, t=_gs: t[:, ko, :]

                    gr_n = gp.tile([P, KO, N], F16, tag=f"gr{par}", name=f"gr{b}_{k}")
                    gi_n = gp.tile([P, KO, N], F16, tag=f"gi{par}", name=f"gi{b}_{k}")
                    gs_n = (
                        None
                        if last
                        else gp.tile([P, KO, N], F16, tag=f"gs{par}", name=f"gs{b}_{k}")
                    )

                    for m in range(KO):
                        msl = slice(m * P, (m + 1) * P)
                        t1 = ps.tile([P, N], F32, tag="t1", name=f"t1_{b}_{k}_{m}")
                        t2 = ps.tile([P, N], F32, tag="t2", name=f"t2_{b}_{k}_{m}")
                        t3 = ps.tile([P, N], F32, tag="t3", name=f"t3_{b}_{k}_{m}")

                        for ko in range(KO):
                            nc.tensor.matmul(
                                t1[:],
                                lhsT=xr_t[:, ko, msl],
                                rhs=rhs_r(ko),
                                start=(ko == 0),
                                stop=(ko == KO - 1),
                            )
                        for ko in range(KO):
                            nc.tensor.matmul(
                                t2[:],
                                lhsT=xi_t[:, ko, msl],
                                rhs=rhs_i(ko),
                                start=(ko == 0),
                                stop=(ko == KO - 1),
                            )
                        for ko in range(KO):
                            nc.tensor.matmul(
                                t3[:],
                                lhsT=xs_t[:, ko, msl],
                                rhs=rhs_s(ko),
                                start=(ko == 0),
                                stop=(ko == KO - 1),
                            )

                        # ScalarE evacuates PSUM -> SBUF fp16 (closest engine
                        # to PSUM; otherwise idle).
                        t1s = sp.tile([P, N], F16, tag="t1s", name=f"t1s_{b}_{k}_{m}")
                        t2s = sp.tile([P, N], F16, tag="t2s", name=f"t2s_{b}_{k}_{m}")
                        t3s = sp.tile([P, N], F16, tag="t3s", name=f"t3s_{b}_{k}_{m}")
                        nc.scalar.copy(t1s[:], t1[:])
                        nc.scalar.copy(t2s[:], t2[:])
                        nc.scalar.copy(t3s[:], t3[:])

                        # VectorE combine: all-fp16 SBUF ops (DVE 2x mode).
                        v1 = vp.tile([P, N], F16, tag="v1", name=f"v1_{b}_{k}_{m}")
                        v2 = vp.tile([P, N], F16, tag="v2", name=f"v2_{b}_{k}_{m}")
                        w2 = vp.tile([P, N], F16, tag="w2", name=f"w2_{b}_{k}_{m}")
                        nc.vector.tensor_sub(v1[:], t1s[:], t2s[:])
                        nc.vector.tensor_add(gr_n[:, m, :], v1[:], ct_r[:, j, m, :])
                        nc.vector.tensor_sub(v2[:], t3s[:], t1s[:])
                        nc.vector.tensor_sub(w2[:], v2[:], t2s[:])
                        nc.vector.tensor_add(gi_n[:, m, :], w2[:], ct_i[:, j, m, :])
                        if last:
                            nc.sync.dma_start(out=or_d[b, m], in_=gr_n[:, m, :])
                            nc.sync.dma_start(out=oi_d[b, m], in_=gi_n[:, m, :])
                        else:
                            nc.vector.tensor_add(
                                gs_n[:, m, :], gr_n[:, m, :], gi_n[:, m, :]
                            )

                    gcur[b] = (gr_n, gi_n, gs_n)

    nc.finalize()
    return nc


def _get_nc() -> bass.Bass:
    if "nc" not in _NC_CACHE:
        _NC_CACHE["nc"] = _build_nc()
    return _NC_CACHE["nc"]


def _tile_layout(m: np.ndarray) -> np.ndarray:
    """[N, N] row-major -> [P, KO, N] (row r at [r % P, r // P, :])."""
    return np.ascontiguousarray(m.reshape(KO, P, N).transpose(1, 0, 2))


def _prep_inputs(x: np.ndarray, coeffs: np.ndarray):
    """Host-side prep: tile/transpose into the DRAM layouts the kernel wants."""
    x = np.ascontiguousarray(x, dtype=np.float32)
    coeffs = np.ascontiguousarray(coeffs, dtype=np.float32)

    # [B, P, KO, N] fp16
    xr = x[:, 0].reshape(B, KO, P, N).transpose(0, 2, 1, 3).astype(np.float16)
    xi = x[:, 1].reshape(B, KO, P, N).transpose(0, 2, 1, 3).astype(np.float16)
    xs = (
        (x[:, 0] + x[:, 1]).reshape(B, KO, P, N).transpose(0, 2, 1, 3)
    ).astype(np.float16)
    xr = np.ascontiguousarray(xr)
    xi = np.ascontiguousarray(xi)
    xs = np.ascontiguousarray(xs)

    crT = coeffs[:, 0].transpose(0, 2, 1)  # [DEG, N, N]
    ciT = coeffs[:, 1].transpose(0, 2, 1)
    ctr = np.empty((DEG - 1, P, KO, N), dtype=np.float16)
    cti = np.empty((DEG - 1, P, KO, N), dtype=np.float16)
    for jj in range(DEG - 1):
        k = DEG - 2 - jj
        ctr[jj] = _tile_layout(crT[k]).astype(np.float16)
        cti[jj] = _tile_layout(ciT[k]).astype(np.float16)
    g0 = np.empty((3, P, KO, N), dtype=np.float16)
    g0[0] = _tile_layout(crT[DEG - 1]).astype(np.float16)
    g0[1] = _tile_layout(ciT[DEG - 1]).astype(np.float16)
    g0[2] = _tile_layout(crT[DEG - 1] + ciT[DEG - 1]).astype(np.float16)

    in_maps = []
    for c in range(NCORES):
        sl = slice(c * BPC, (c + 1) * BPC)
        in_maps.append(
            {
                "xr": np.ascontiguousarray(xr[sl]),
                "xi": np.ascontiguousarray(xi[sl]),
                "xs": np.ascontiguousarray(xs[sl]),
                "ctr": ctr,
                "cti": cti,
                "g0": g0,
            }
        )
    return in_maps


def _assemble_output(results) -> np.ndarray:
    out = np.empty((B, 2, N, N), dtype=np.float32)
    for c in range(NCORES):
        o_r = results[c]["o_r"].reshape(BPC, N, N).astype(np.float32)
        o_i = results[c]["o_i"].reshape(BPC, N, N).astype(np.float32)
        for b in range(BPC):
            out[c * BPC + b, 0] = o_r[b].T
            out[c * BPC + b, 1] = o_i[b].T
    return out


def run_sharded(x: np.ndarray, coeffs: np.ndarray, **run_kwargs):
    """Run the SPMD kernel on 8 cores; returns (output, BassKernelResults)."""
    nc = _get_nc()
    in_maps = _prep_inputs(x, coeffs)
    res = run_bass_kernel_spmd(nc, in_maps, list(range(NCORES)), **run_kwargs)
    return _assemble_output(res.results), res


def kernel(x: np.ndarray, coeffs: np.ndarray) -> np.ndarray:
    out, _ = run_sharded(x, coeffs)
    return out
